# revision 39
# baseline (speedup 1.0000x reference)
"""Trainium2 Bass kernel for nn_Attention_30554397344218.

Multi-head attention (B=8, S=1040, D=1024, H=16, hd=64) with 2D vision RoPE
on the 1024 grid tokens after a 16-token puzzle prefix.

Sharding: pure data-parallel - one batch element per NeuronCore (8 cores,
no collectives); weights broadcast; host gathers the 8 outputs.

v2 design, 375.7us -> 319.8us (all-bf16 matmuls; fp8 DoubleRow was tested
and rejected: rel err 2.4-5.2e-2 vs the 2e-2 gate because softmax does not
damp relative error - the attention output's signal shrinks with its noise):
  - phase 1: q,k projections in transposed layout (head_dim on partitions,
    2 heads per 128-chunk; k duplicated with one head zeroed so score
    matmuls contract a full K=128), RoPE via PE permutation matmul + DVE
    mul/add, software-pipelined. Inputs land via 2-3 large DMAs per tensor
    split across the three hw DMA queues (~105 GB/s each) in need-order
    (xt+wq thirds first); 64 warmup matmuls cover the ramp. v j-tiles 0-2
    run on phase-1 psum to hide the psum pool turnover.
  - phase 2 is ACT-exp-bound (94% ACT occupancy; 1114ns per 1024-wide
    exp is the hw rate): the exp stream (8 j-tile exps + 1 strided tails
    exp per head) runs against scores, att@v of head h-2 (lag-2
    pipeline, 3 pt ring buffers, chunks emitted in two halves around the
    next scores matmul so the exp lookahead never drains), the rest of
    the V projection (woven under heads 0-1), per-chunk normalization,
    and head 14's att@v inside head 15's window.
  - j8 (the 16-row key-tail tile) is packed 3 heads per [128,1024] psum
    tile at partition offsets 0/32/64 (AP base limit), one exp per
    group instead of one full-width exp per head (-12us ACT); v8's 16
    rows are replicated at those offsets so the att@v j8 stationary
    shares the moving tile's partition base (codegen requirement).
  - scores PSUM: [128,1024] 2-bank tiles + one shared tail bank for all
    16-wide query-tail strips -> 8 banks total with st 2x2 + tails 1 +
    po 3 (att@v out / norm / v-proj share the po pool; the 3rd po slot
    was worth 7us of att@v slot-chain slack).
  - normalization selector matmuls in bf16: the old fp32 ones forced the
    PE into a half-clock mode (HAM k=4) for ~48us, slowing interleaved
    bf16 matmuls from 379ns to 634ns per 512 columns.
  - otc[c] reuses qr[c]'s pool slot (qr[c]'s last reader finishes exactly
    before otc[c]'s first write), freeing 16.6KB of SBUF.
  - output projection: it-tiles 0/1 (st slots) and it2 (po slots)
    accumulate chunks 0-6 before chunk 7's reciprocal chain resolves,
    then chunk 7 joins; yps tiles created lazily so the st ring stays
    aligned; y copies alternate ACT/DVE; bf16 output over 3 queues.
  - Note: the device clock is bimodal run-to-run (~320us fast state vs
    ~386us degraded state; NEURON_RT_RESET_CORES=1 recovers a wedged
    device after an NRT fault). Timings here are fast-state.
  - DVE/gpsimd Schraudolph exp offload was tried and reverted: both
    engines' queue latency poisons the st-slot chain (+23us / +35us).
"""

import numpy as np
import ml_dtypes

B, S, D, H, HD = 8, 1040, 1024, 16, 64
PFX = 16
GRID = 32
NCHUNK = 8
NJT = 9
TAIL = S - 8 * 128  # 16
ICH3 = [(0, 512), (512, 512), (1024, 16)]
ICH2 = [(0, 512), (512, 512)]
NPT = 3  # pt ring buffers (lag-2 att@v pipeline)
BF16 = ml_dtypes.bfloat16

_compiled = None


def _rope_tables():
    half, quarter = HD // 2, HD // 4
    frac = 2.0 * np.arange(quarter, dtype=np.float64) / half
    ts = 10000.0 ** frac
    row = np.arange(GRID, dtype=np.float64)[:, None] / ts[None, :]
    row_ang = np.broadcast_to(row[:, None, :], (GRID, GRID, quarter)).reshape(
        GRID * GRID, quarter
    )
    col_ang = np.broadcast_to(row[None, :, :], (GRID, GRID, quarter)).reshape(
        GRID * GRID, quarter
    )
    cos64 = np.concatenate(
        [np.cos(row_ang).T, np.cos(row_ang).T, np.cos(col_ang).T, np.cos(col_ang).T],
        axis=0,
    )
    s64 = np.concatenate(
        [-np.sin(row_ang).T, np.sin(row_ang).T, -np.sin(col_ang).T, np.sin(col_ang).T],
        axis=0,
    )
    cosf = np.ones((HD, S), np.float64)
    sf = np.zeros((HD, S), np.float64)
    cosf[:, PFX:] = cos64
    sf[:, PFX:] = s64
    cos2 = np.concatenate([cosf, cosf], axis=0).astype(BF16)
    s2 = np.concatenate([sf, sf], axis=0).astype(BF16)
    return cos2, s2


def _swap_matrix():
    swp = np.zeros((128, 128), np.float32)
    for i in range(128):
        swp[i, i ^ 16] = 1.0
    return swp.astype(BF16)


def _build_body(nc, tc, tile, mybir, aps):
    from contextlib import ExitStack

    from concourse.alu_op_type import AluOpType

    bf = mybir.dt.bfloat16
    f32 = mybir.dt.float32
    i32 = mybir.dt.int32
    Exp = mybir.ActivationFunctionType.Exp
    # Schraudolph exp constants (folding in the 1/sqrt(HD) score scale):
    # exp(x*0.125) ~= bitcast_f32(int32(A*x + B)), rms err 1.8%
    SCHR_A = float(0.125 * (1 << 23) / np.log(2.0))
    SCHR_B = float(127.0 * (1 << 23) - 486411.0)
    SCHR_JS = ()  # DVE/gpsimd Schraudolph exp offload: tried and reverted - both engines' queue latency poisons the st->pt->att@v chain
    xT, Wq, Wk, Wv, Wo = aps["xT"], aps["Wq"], aps["Wk"], aps["Wv"], aps["Wo"]
    COS2, S2, SWP, SEL2, OUT = (
        aps["COS2"], aps["S2"], aps["SWP"], aps["SEL2"], aps["out"],
    )

    def rows_of(j):
        return 128 if j < 8 else TAIL

    with ExitStack() as ctx:
        # ---- persistent SBUF pools (live across both phases)
        p_tab = ctx.enter_context(tc.tile_pool(name="tab", bufs=1))
        p_xt = ctx.enter_context(tc.tile_pool(name="xt", bufs=1))
        p_wv = ctx.enter_context(tc.tile_pool(name="wv", bufs=1))
        p_qk = ctx.enter_context(tc.tile_pool(name="qk", bufs=24))
        p_vx = ctx.enter_context(tc.tile_pool(name="vx", bufs=9))

        sel2 = p_tab.tile([2, 128], bf, tag="sel2")
        xt = p_xt.tile([128, 8, S], bf, tag="xt")
        wv_t = p_wv.tile([128, 8, D], bf, tag="wv")

        qr = [p_qk.tile([128, S], bf, tag="qk", name=f"qr{i}") for i in range(NCHUNK)]
        # krz is 16 columns wider than S: zeroed key-columns that let the
        # group-j8 score matmuls emit 32-tall strips (16 real keys + 16
        # zero rows), so the packed psum tile needs no gap memset
        krz = [
            [p_qk.tile([128, S + 16], bf, tag="qk", name=f"krz{i}_{z}") for z in range(2)]
            for i in range(NCHUNK)
        ]
        vx = [p_vx.tile([128, 1104], bf, tag="vx", name=f"vx{i}") for i in range(NJT)]
        # otc[c] is created lazily at its first att@v write (head 2c+2's
        # weave): pool rotation then hands it qr[c]'s slot, whose last
        # reader (scores of head 2c+1) is already done - a free 16.6KB.
        otc = [None] * NCHUNK

        # ---- input DMAs, priority-ordered and queue-parallel: each DMA
        # queue moves ~105 GB/s, so the tensors needed first (xt, wq) are
        # split across two queues each; weights stream behind them on the
        # same queues in need-order (wq -> wk -> wv).
        xT3 = xT.rearrange("(k p) s -> p k s", p=128)
        nc.gpsimd.dma_start(out=xt[:, 0:3, :], in_=xT3[:, 0:3, :])
        nc.sync.dma_start(out=xt[:, 3:6, :], in_=xT3[:, 3:6, :])
        nc.scalar.dma_start(out=xt[:, 6:8, :], in_=xT3[:, 6:8, :])

        # ================= phase 1: q/k projections + RoPE =================
        with ExitStack() as p1:
            p_t1 = p1.enter_context(tc.tile_pool(name="t1", bufs=1))
            p_w = p1.enter_context(tc.tile_pool(name="w", bufs=2))
            p_tmp = p1.enter_context(tc.tile_pool(name="tmp", bufs=3))
            p_ps1 = p1.enter_context(tc.tile_pool(name="ps1", bufs=6, space="PSUM"))
            p_ps2 = p1.enter_context(tc.tile_pool(name="ps2", bufs=2, space="PSUM"))

            cos_sb = p_t1.tile([128, S], bf, tag="cos")
            s_sb = p_t1.tile([128, S], bf, tag="sin")
            swp_sb = p_t1.tile([128, 128], bf, tag="swp")
            wq_t = p_w.tile([128, 8, D], bf, tag="w", name="wq")
            wk_t = p_w.tile([128, 8, D], bf, tag="w", name="wk")
            Wq3 = Wq.rearrange("(k p) m -> p k m", p=128)
            Wk3 = Wk.rearrange("(k p) m -> p k m", p=128)
            Wv3 = Wv.rearrange("(k p) m -> p k m", p=128)
            nc.gpsimd.dma_start(out=wq_t[:, 0:3, :], in_=Wq3[:, 0:3, :])
            nc.sync.dma_start(out=wq_t[:, 3:6, :], in_=Wq3[:, 3:6, :])
            nc.scalar.dma_start(out=wq_t[:, 6:8, :], in_=Wq3[:, 6:8, :])
            nc.scalar.dma_start(out=swp_sb, in_=SWP[:, :])
            nc.scalar.dma_start(out=sel2, in_=SEL2[:, :])
            nc.gpsimd.dma_start(out=cos_sb, in_=COS2[:, :])
            nc.gpsimd.dma_start(out=s_sb, in_=S2[:, :])
            nc.gpsimd.dma_start(out=wk_t[:, 0:4, :], in_=Wk3[:, 0:4, :])
            nc.scalar.dma_start(out=wk_t[:, 4:8, :], in_=Wk3[:, 4:8, :])
            nc.gpsimd.dma_start(out=wv_t[:, 0:4, :], in_=Wv3[:, 0:4, :])
            nc.scalar.dma_start(out=wv_t[:, 4:8, :], in_=Wv3[:, 4:8, :])

            # PE warmup: scratch matmuls bring the clock up while DMAs land
            wa = p_tmp.tile([128, 512], bf, tag="wa", bufs=1)
            wb = p_tmp.tile([128, 128], bf, tag="wb", bufs=1)
            nc.vector.memset(wa, 0.0)
            nc.vector.memset(wb, 0.0)
            wps = p_ps1.tile([128, 512], f32, tag="mm1", name="warm_ps")
            for _w in range(64):
                nc.tensor.matmul(wps, wb, wa, start=True, stop=True)

            # memsets on the (otherwise idle) DVE; gpsimd only issues DMAs
            for c in range(NCHUNK):
                nc.vector.memset(krz[c][0][64:128, :], 0.0)
                nc.vector.memset(krz[c][1][0:64, :], 0.0)
                nc.gpsimd.memset(krz[c][0][0:64, 1040:1056], 0.0)
                nc.gpsimd.memset(krz[c][1][64:128, 1040:1056], 0.0)
            for j in range(NJT):
                r = rows_of(j)
                vx3 = vx[j][:, :1040].rearrange("p (h d) -> p h d", d=65)
                nc.gpsimd.memset(vx[j][:, 1040:1104], 0.0)
                nc.gpsimd.memset(vx3[:r, :, 64:65], 1.0)
                if j == 8:
                    # v8's 16 rows are replicated at partition offsets 32
                    # and 64 so the att@v j8 stationary can match the
                    # packed gpt moving tile's partition base
                    nc.gpsimd.memset(vx3[32 : 32 + r, :, 64:65], 1.0)
                    nc.gpsimd.memset(vx3[64 : 64 + r, :, 64:65], 1.0)
            # preload the exp ACT table so phase 2 doesn't pay the switch
            nc.scalar.activation(wa[0:1, 0:8], wa[0:1, 0:8], Exp, scale=0.0)

            def emit_mm1(which, w_t, c):
                raw = p_tmp.tile([128, S], bf, tag="raw", name=f"raw_{which}{c}")
                pss = [
                    p_ps1.tile([128, 512], f32, tag="mm1", name=f"mm1_{which}{c}_{i}")
                    for i in range(3)
                ]
                for k in range(8):
                    for i, (off, wdt) in enumerate(ICH3):
                        nc.tensor.matmul(
                            pss[i][:, :wdt],
                            w_t[:, k : k + 1, c * 128 : (c + 1) * 128],
                            xt[:, k : k + 1, off : off + wdt],
                            start=(k == 0),
                            stop=(k == 7),
                        )
                for i, (off, wdt) in enumerate(ICH3):
                    nc.scalar.copy(raw[:, off : off + wdt], pss[i][:, :wdt])
                return raw

            def emit_rope(which, c, raw):
                for off, wdt in ICH3:
                    sw = p_ps2.tile([128, 512], f32, tag="swp")
                    nc.tensor.matmul(
                        sw[:, :wdt], swp_sb, raw[:, off : off + wdt],
                        start=True, stop=True,
                    )
                    t2 = p_tmp.tile([128, 512], bf, tag="t2")
                    nc.vector.tensor_mul(
                        t2[:, :wdt], sw[:, :wdt], s_sb[:, off : off + wdt]
                    )
                    t1 = p_tmp.tile([128, 512], bf, tag="t1")
                    nc.vector.tensor_mul(
                        t1[:, :wdt], raw[:, off : off + wdt],
                        cos_sb[:, off : off + wdt],
                    )
                    if which == "q":
                        nc.vector.tensor_add(
                            qr[c][:, off : off + wdt], t1[:, :wdt], t2[:, :wdt]
                        )
                    else:
                        nc.vector.tensor_add(
                            krz[c][0][0:64, off : off + wdt],
                            t1[0:64, :wdt], t2[0:64, :wdt],
                        )
                        nc.vector.tensor_add(
                            krz[c][1][64:128, off : off + wdt],
                            t1[64:128, :wdt], t2[64:128, :wdt],
                        )

            steps = [("q", c) for c in range(NCHUNK)] + [
                ("k", c) for c in range(NCHUNK)
            ]
            pending = None
            for which, c in steps:
                raw = emit_mm1(which, wq_t if which == "q" else wk_t, c)
                if pending is not None:
                    emit_rope(*pending)
                pending = (which, c, raw)

            emit_rope(*pending)
            # v tiles 0-2 on phase-1 PSUM: PE work that overlaps the last
            # rope's DVE tail and the psum pool turnover barrier
            for j in range(3):
                r = rows_of(j)
                vx3 = vx[j][:, :1040].rearrange("p (h d) -> p h d", d=65)
                for ci in range(2):
                    psv = p_ps1.tile(
                        [128, 512], f32, tag="mm1", name=f"pv1_{j}_{ci}"
                    )
                    for k in range(8):
                        nc.tensor.matmul(
                            psv[:r, :],
                            xt[:, k : k + 1, j * 128 : j * 128 + r],
                            wv_t[:, k : k + 1, ci * 512 : (ci + 1) * 512],
                            start=(k == 0),
                            stop=(k == 7),
                        )
                    nc.vector.tensor_copy(
                        vx3[:r, ci * 8 : (ci + 1) * 8, 0:64],
                        psv[:r, :].rearrange("p (h d) -> p h d", h=8),
                    )

        # ============ phase 2: v-proj + attention (ACT-exp paced) ==========
        with ExitStack() as p2:
            p_wo = p2.enter_context(tc.tile_pool(name="wo", bufs=1))
            p_pt = p2.enter_context(tc.tile_pool(name="pt", bufs=NPT))
            p_cg = p2.enter_context(tc.tile_pool(name="cg", bufs=2))
            p_cs = p2.enter_context(tc.tile_pool(name="cs", bufs=2))
            p_rc = p2.enter_context(tc.tile_pool(name="rc", bufs=2))
            p_y = p2.enter_context(tc.tile_pool(name="y", bufs=2))
            p_st = p2.enter_context(tc.tile_pool(name="st", bufs=2, space="PSUM"))
            p_tl = p2.enter_context(tc.tile_pool(name="tl", bufs=1, space="PSUM"))
            p_po = p2.enter_context(tc.tile_pool(name="po", bufs=3, space="PSUM"))

            wo_t = p_wo.tile([128, 8, D], bf, tag="wo")
            Wo3 = Wo.rearrange("(k p) m -> p k m", p=128)
            nc.sync.dma_start(out=wo_t[:, 0:4, :], in_=Wo3[:, 0:4, :])
            nc.sync.dma_start(out=wo_t[:, 4:8, :], in_=Wo3[:, 4:8, :])

            pt = [
                p_pt.tile([128, 8 * S], bf, tag="pt", name=f"pt{i}")
                for i in range(NPT)
            ]
            # j8 (the 16-key tail tile) is handled per 4-head group: the 4
            # heads' [16,1040] score strips sit at partition offsets 0/32/
            # 64/96 of one shared tile so ONE exp covers all of them
            # (per-head j8 exps cost a full 1095ns for 16 rows each).
            p_gpt = p2.enter_context(tc.tile_pool(name="gpt", bufs=2))
            gpt = [
                p_gpt.tile([128, S], bf, tag="gpt", name=f"gpt{g}")
                for g in range(6)
            ]
            cs = [None] * NCHUNK  # per-chunk [2,S] denominator tiles
            p_i32 = p2.enter_context(tc.tile_pool(name="i32", bufs=1))
            # tl: [0:128) tails j0-7 (even head), [128:256) odd head,
            # [256:352) the 6 groups' j8 query-tails
            tl = p_tl.tile([128, 352], f32, tag="tl")
            nc.vector.memset(tl[:, 256:352], 0.0)

            def emit_v_tile(j):
                r = rows_of(j)
                vx3 = vx[j][:, :1040].rearrange("p (h d) -> p h d", d=65)
                for ci in range(2):
                    psv = p_po.tile([128, 512], f32, tag="po", name=f"pv{j}_{ci}")
                    for k in range(8):
                        nc.tensor.matmul(
                            psv[:r, :],
                            xt[:, k : k + 1, j * 128 : j * 128 + r],
                            wv_t[:, k : k + 1, ci * 512 : (ci + 1) * 512],
                            start=(k == 0),
                            stop=(k == 7),
                        )
                    bases = (0, 32, 64) if j == 8 else (0,)
                    for bs in bases:
                        nc.vector.tensor_copy(
                            vx3[bs : bs + r, ci * 8 : (ci + 1) * 8, 0:64],
                            psv[:r, :].rearrange("p (h d) -> p h d", h=8),
                        )

            def emit_scores_j(h, j):
                c, hb = divmod(h, 2)
                ptf = pt[h % NPT]
                tb = (h % 2) * 128
                st = p_st.tile([128, 1024], f32, tag="st", name=f"st{h}_{j}")
                for off, wdt in ICH2:
                    nc.tensor.matmul(
                        st[:, off : off + wdt],
                        krz[c][hb][:, j * 128 : (j + 1) * 128],
                        qr[c][:, off : off + wdt],
                        start=True,
                        stop=True,
                    )
                nc.tensor.matmul(
                    tl[:, tb + j * 16 : tb + (j + 1) * 16],
                    krz[c][hb][:, j * 128 : (j + 1) * 128],
                    qr[c][:, 1024:1040],
                    start=True,
                    stop=True,
                )
                if j in SCHR_JS:
                    # DVE Schraudolph exp: offloads the ACT engine, which
                    # paces the whole attention phase
                    it = p_i32.tile([128, 1024], i32, tag="i32", name=f"i{h}_{j}")
                    nc.vector.tensor_scalar(
                        it, st[:, :], SCHR_A, SCHR_B,
                        AluOpType.mult, AluOpType.add,
                    )
                    nc.gpsimd.tensor_copy(
                        ptf[:, j * S : j * S + 1024], it.bitcast(f32)
                    )
                else:
                    nc.scalar.activation(
                        ptf[:, j * S : j * S + 1024], st[:, :],
                        Exp, scale=1.0 / np.sqrt(HD),
                    )

            def emit_tails_exp(h):
                ptf = pt[h % NPT]
                ptv = ptf.rearrange("p (j q) -> p j q", q=S)
                tb = (h % 2) * 128
                nc.scalar.activation(
                    ptv[:, 0:8, 1024:1040],
                    tl[:, tb : tb + 128].rearrange("p (j t) -> p j t", t=16),
                    Exp,
                    scale=1.0 / np.sqrt(HD),
                )

            def emit_group_j8(g):
                # scores + exp for the j8 key tile of heads 3g..3g+2, packed
                # at partition offsets 32m (AP base must be 0/32/64); the
                # 32-wide stationary (16 real + 16 zero key columns) writes
                # full 32-tall strips, and rows 96:128 are never read, so
                # the packed tile needs no memset
                stg = p_st.tile([128, 1024], f32, tag="st", name=f"stg{g}")
                for m in range(3):
                    hh = 3 * g + m
                    if hh >= H:
                        break
                    c, hb = divmod(hh, 2)
                    for off, wdt in ICH2:
                        nc.tensor.matmul(
                            stg[32 * m : 32 * m + 32, off : off + wdt],
                            krz[c][hb][:, 1024:1056],
                            qr[c][:, off : off + wdt],
                            start=True,
                            stop=True,
                        )
                    nc.tensor.matmul(
                        tl[32 * m : 32 * m + 32, 256 + g * 16 : 272 + g * 16],
                        krz[c][hb][:, 1024:1056],
                        qr[c][:, 1024:1040],
                        start=True,
                        stop=True,
                    )
                nh = min(3, H - 3 * g)
                nc.scalar.activation(
                    gpt[g][0 : 32 * nh, 0:1024],
                    stg[0 : 32 * nh, :],
                    Exp,
                    scale=1.0 / np.sqrt(HD),
                )
                nc.scalar.activation(
                    gpt[g][0 : 32 * nh, 1024:1040],
                    tl[0 : 32 * nh, 256 + g * 16 : 272 + g * 16],
                    Exp,
                    scale=1.0 / np.sqrt(HD),
                )

            cstage = {}
            otd = {}

            def emit_attv_part1(h, i):
                # first half of an att@v chunk's key accumulation; split so
                # the PE block between two scores matmuls stays short enough
                # that the ACT exp stream never drains its lookahead
                c, hb = divmod(h, 2)
                off, wdt = ICH3[i]
                ptf = pt[h % NPT]
                if otc[c] is None:
                    otc[c] = p_qk.tile([128, S], bf, tag="qk", name=f"otc{c}")
                if i == 0:
                    cstage[h] = p_cg.tile([65, S], f32, tag="cg", name=f"cst{h}")
                ot = p_po.tile([128, 512], f32, tag="po", name=f"ot{h}_{i}")
                otd[(h, i)] = ot
                for j in range(4):
                    nc.tensor.matmul(
                        ot[:, :wdt],
                        vx[j][:, h * 65 : h * 65 + 128],
                        ptf[:, j * S + off : j * S + off + wdt],
                        start=(j == 0),
                        stop=False,
                    )

            def emit_attv_part2(h, i):
                c, hb = divmod(h, 2)
                off, wdt = ICH3[i]
                ptf = pt[h % NPT]
                ot = otd.pop((h, i))
                for j in range(4, 8):
                    nc.tensor.matmul(
                        ot[:, :wdt],
                        vx[j][:, h * 65 : h * 65 + 128],
                        ptf[:, j * S + off : j * S + off + wdt],
                        start=False,
                        stop=False,
                    )
                m = h % 3
                nc.tensor.matmul(
                    ot[:, :wdt],
                    vx[8][32 * m : 32 * m + TAIL, h * 65 : h * 65 + 128],
                    gpt[h // 3][32 * m : 32 * m + TAIL, off : off + wdt],
                    start=False,
                    stop=True,
                )
                nc.vector.tensor_copy(
                    otc[c][hb * 64 : hb * 64 + 64, off : off + wdt],
                    ot[0:64, :wdt],
                )
                nc.vector.tensor_copy(
                    cstage[h][64:65, off : off + wdt], ot[64:65, :wdt]
                )
                if i == 2:
                    if cs[c] is None:
                        cs[c] = p_cs.tile([2, S], f32, tag="cs", name=f"cs{c}")
                    (nc.sync if h % 2 == 0 else nc.gpsimd).dma_start(
                        out=cs[c][hb : hb + 1, :], in_=cstage[h][64:65, :]
                    )

            def emit_attv_chunk(h, i):
                emit_attv_part1(h, i)
                emit_attv_part2(h, i)

            def emit_norm(c):
                rcp = p_rc.tile([2, S], f32, tag="rcp", name=f"rcp{c}")
                nc.vector.reciprocal_approx_fast(rcp, cs[c])
                rcpb = p_rc.tile([2, S], bf, tag="rcpb", name=f"rcpb{c}")
                nc.vector.tensor_copy(rcpb, rcp)
                for off, wdt in ICH3:
                    psb = p_po.tile([128, 512], f32, tag="po", name=f"nm{c}_{off}")
                    nc.tensor.matmul(
                        psb[:, :wdt], sel2, rcpb[:, off : off + wdt],
                        start=True, stop=True,
                    )
                    nc.vector.tensor_mul(
                        otc[c][:, off : off + wdt],
                        otc[c][:, off : off + wdt],
                        psb[:, :wdt],
                    )

            # phase 1.5: rest of V projection woven under heads 0-1
            vq = [3, 4, 5, 6, 7]
            for h in (0, 1):
                for j in range(8):
                    emit_scores_j(h, j)
                    if j in (1, 3, 5) and vq:
                        emit_v_tile(vq.pop(0))
                emit_tails_exp(h)
                if h % 3 == 0:
                    emit_group_j8(h // 3)
            emit_v_tile(8)

            # steady state: head h scores woven with att@v of head h-2;
            # h15 additionally absorbs att@v(14) (its exps are done by then)
            # att@v chunks are emitted in two halves around the next scores
            # matmul so the PE block between score pairs stays short and the
            # ACT exp stream keeps its lookahead fed
            for h in range(2, H):
                for j in range(8):
                    emit_scores_j(h, j)
                    if h < 15:
                        if j in (1, 4, 6):
                            emit_attv_part1(h - 2, {1: 0, 4: 1, 6: 2}[j])
                        if j in (2, 5, 7):
                            emit_attv_part2(h - 2, {2: 0, 5: 1, 7: 2}[j])
                    else:
                        if j in (1, 4, 6):
                            emit_attv_chunk(h - 2, {1: 0, 4: 1, 6: 2}[j])
                        if j in (2, 5, 7):
                            emit_attv_chunk(14, {2: 0, 5: 1, 7: 2}[j])
                emit_tails_exp(h)
                if h % 3 == 0:
                    emit_group_j8(h // 3)
                # norm(c) needs att@v(2c+1), complete at the end of head
                # 2c+3's window -> emit at h = 2c+4; norm(6) right after
                # att@v(13) lands inside h15
                if h >= 4 and h % 2 == 0:
                    emit_norm((h - 4) // 2)
                if h == 15:
                    emit_norm(6)

            # drain: att@v for head 15, then norm(7)
            for i in range(3):
                emit_attv_chunk(15, i)

            def emit_yproj_mm(it, cs_, start, stop):
                r = rows_of(it)
                if it not in yps:
                    yps[it] = p_st.tile([128, 1024], f32, tag="st", name=f"y{it}")
                for ci in range(2):
                    for c in cs_:
                        nc.tensor.matmul(
                            yps[it][:r, ci * 512 : (ci + 1) * 512],
                            otc[c][:, it * 128 : it * 128 + r],
                            wo_t[:, c : c + 1, ci * 512 : (ci + 1) * 512],
                            start=(start and c == cs_[0]),
                            stop=(stop and c == cs_[-1]),
                        )

            def emit_yproj_out(it):
                r = rows_of(it)
                for ci in range(2):
                    ysb = p_y.tile([128, 512], bf, tag="ysb")
                    eng = (nc.scalar.copy, nc.vector.tensor_copy)[ci]
                    eng(ysb[:r, :], yps[it][:r, ci * 512 : (ci + 1) * 512])
                    qs = (
                        (nc.sync, nc.gpsimd)[ci]
                        if it >= 7
                        else (nc.sync, nc.gpsimd, nc.scalar)[(2 * it + ci) % 3]
                    )
                    qs.dma_start(
                        out=OUT[it * 128 : it * 128 + r, ci * 512 : (ci + 1) * 512],
                        in_=ysb[:r, :],
                    )

            # output projection: it0/it1 accumulate chunks 0-6 BEFORE
            # norm(7) is emitted, hiding the chunk-7 reciprocal chain
            # (cstage DMA -> rcp -> selector matmul -> DVE mul) behind PE
            # work; chunk 7 joins as the final accumulation step.
            # yps tiles are created lazily so the st-slot ring only
            # contains tiles actually used (it2 runs on po slots; an unused
            # yps[2] would misalign the ring and stall it3 behind it1)
            yps = {}
            emit_yproj_mm(0, list(range(7)), True, False)
            emit_yproj_mm(1, list(range(7)), True, False)
            # it2's partial runs on po slots (both st slots are held open by
            # it0/it1) to keep the PE fed through norm(7)'s reciprocal chain
            yp2 = [
                p_po.tile([128, 512], f32, tag="po", name=f"yp2_{ci}")
                for ci in range(2)
            ]
            for ci in range(2):
                for c in range(7):
                    nc.tensor.matmul(
                        yp2[ci][:, :],
                        otc[c][:, 2 * 128 : 3 * 128],
                        wo_t[:, c : c + 1, ci * 512 : (ci + 1) * 512],
                        start=(c == 0),
                        stop=False,
                    )
            emit_norm(7)
            emit_yproj_mm(0, [7], False, True)
            emit_yproj_out(0)
            emit_yproj_mm(1, [7], False, True)
            emit_yproj_out(1)
            for ci in range(2):
                nc.tensor.matmul(
                    yp2[ci][:, :],
                    otc[7][:, 2 * 128 : 3 * 128],
                    wo_t[:, 7:8, ci * 512 : (ci + 1) * 512],
                    start=False,
                    stop=True,
                )
                ysb = p_y.tile([128, 512], bf, tag="ysb")
                (nc.scalar.copy, nc.vector.tensor_copy)[ci](ysb, yp2[ci][:, :])
                (nc.sync, nc.gpsimd)[ci].dma_start(
                    out=OUT[2 * 128 : 3 * 128, ci * 512 : (ci + 1) * 512],
                    in_=ysb,
                )
            for it in range(3, NJT):
                emit_yproj_mm(it, list(range(8)), True, True)
                emit_yproj_out(it)


def _build():
    global _compiled
    if _compiled is not None:
        return _compiled
    import concourse.bass as bass  # noqa: F401
    import concourse.mybir as mybir
    import concourse.tile as tile
    from concourse import bacc

    nc = bacc.Bacc("TRN2", target_bir_lowering=False, debug=False)
    bf = mybir.dt.bfloat16
    aps = {
        "xT": nc.dram_tensor("xT", [D, S], bf, kind="ExternalInput").ap(),
        "Wq": nc.dram_tensor("Wq", [D, H * HD], bf, kind="ExternalInput").ap(),
        "Wk": nc.dram_tensor("Wk", [D, H * HD], bf, kind="ExternalInput").ap(),
        "Wv": nc.dram_tensor("Wv", [D, H * HD], bf, kind="ExternalInput").ap(),
        "Wo": nc.dram_tensor("Wo", [H * HD, D], bf, kind="ExternalInput").ap(),
        "COS2": nc.dram_tensor("COS2", [128, S], bf, kind="ExternalInput").ap(),
        "S2": nc.dram_tensor("S2", [128, S], bf, kind="ExternalInput").ap(),
        "SWP": nc.dram_tensor("SWP", [128, 128], bf, kind="ExternalInput").ap(),
        "SEL2": nc.dram_tensor("SEL2", [2, 128], bf, kind="ExternalInput").ap(),
        "out": nc.dram_tensor("out", [S, D], bf, kind="ExternalOutput").ap(),
    }
    with tile.TileContext(nc) as tc:
        _build_body(nc, tc, tile, mybir, aps)
    nc.compile()
    _compiled = nc
    return nc


def _install_trace_shim():
    """The agent image's antenv lacks axon_hooks, so run_bass_kernel_spmd's
    trace path can't find the NTFF profile hook trn_boot would have set.
    Recreate the module and install the ctypes hook; skip the S3 artifact
    upload (no creds needed for local timing)."""
    import sys
    import types

    if "antenv.axon_hooks" not in sys.modules:
        import antenv  # noqa: F401

        mod = types.ModuleType("antenv.axon_hooks")
        mod._hook = None

        def set_axon_ntff_profile_hook(h):
            mod._hook = h

        def get_axon_ntff_profile_hook():
            return mod._hook

        mod.set_axon_ntff_profile_hook = set_axon_ntff_profile_hook
        mod.get_axon_ntff_profile_hook = get_axon_ntff_profile_hook
        sys.modules["antenv.axon_hooks"] = mod

    import antenv.axon_hooks as ah

    if ah.get_axon_ntff_profile_hook() is None:
        from trn_agent_boot.trn_boot import _ntff_profile_via_ctypes

        ah.set_axon_ntff_profile_hook(
            _ntff_profile_via_ctypes("/opt/axon/libaxon_pjrt.so")
        )

    import concourse.bass_utils as bu

    bu.upload_artifacts = lambda tmpdir: f"local://{tmpdir}"


def run(inputs, trace=False):
    """Returns (output (8,1040,1024) f32, exec_time_ns or None)."""
    if trace:
        _install_trace_shim()
    from concourse.bass_utils import run_bass_kernel_spmd

    nc = _build()
    x = np.asarray(inputs["x"], np.float32)
    wq = np.asarray(inputs["Wq"], np.float32).reshape(D, H * HD).astype(BF16)
    wk = np.asarray(inputs["Wk"], np.float32).reshape(D, H * HD).astype(BF16)
    wv = np.asarray(inputs["Wv"], np.float32).reshape(D, H * HD).astype(BF16)
    wo = np.asarray(inputs["Wo"], np.float32).reshape(H * HD, D).astype(BF16)
    cos2, s2 = _rope_tables()
    swp = _swap_matrix()
    sel2b = np.zeros((2, 128), np.float32)
    sel2b[0, 0:64] = 1.0
    sel2b[1, 64:128] = 1.0
    shared = {
        "Wq": wq, "Wk": wk, "Wv": wv, "Wo": wo,
        "COS2": cos2, "S2": s2, "SWP": swp, "SEL2": sel2b.astype(BF16),
    }
    in_maps = [
        dict(shared, xT=np.ascontiguousarray(x[b].T).astype(BF16)) for b in range(B)
    ]
    res = run_bass_kernel_spmd(nc, in_maps, core_ids=list(range(B)), trace=trace)
    out = np.stack([np.asarray(r["out"], np.float32) for r in res.results], axis=0)
    return out, res.exec_time_ns


def kernel(x, Wq, Wk, Wv, Wo):
    out, _ = run({"x": x, "Wq": Wq, "Wk": Wk, "Wv": Wv, "Wo": Wo})
    return out


# revision 40
# speedup vs baseline: 1.0091x; 1.0091x over previous
"""Trainium2 Bass kernel for nn_Attention_30554397344218.

Multi-head attention (B=8, S=1040, D=1024, H=16, hd=64) with 2D vision RoPE
on the 1024 grid tokens after a 16-token puzzle prefix.

Sharding: pure data-parallel - one batch element per NeuronCore (8 cores,
no collectives); weights broadcast; host gathers the 8 outputs.

v2 design, 375.7us -> 319.8us (all-bf16 matmuls; fp8 DoubleRow was tested
and rejected: rel err 2.4-5.2e-2 vs the 2e-2 gate because softmax does not
damp relative error - the attention output's signal shrinks with its noise):
  - phase 1: q,k projections in transposed layout (head_dim on partitions,
    2 heads per 128-chunk; k duplicated with one head zeroed so score
    matmuls contract a full K=128), RoPE via PE permutation matmul + DVE
    mul/add, software-pipelined. Inputs land via 2-3 large DMAs per tensor
    split across the three hw DMA queues (~105 GB/s each) in need-order
    (xt+wq thirds first); 64 warmup matmuls cover the ramp. v j-tiles 0-2
    run on phase-1 psum to hide the psum pool turnover.
  - phase 2 is ACT-exp-bound (94% ACT occupancy; 1114ns per 1024-wide
    exp is the hw rate): the exp stream (8 j-tile exps + 1 strided tails
    exp per head) runs against scores, att@v of head h-2 (lag-2
    pipeline, 3 pt ring buffers, chunks emitted in two halves around the
    next scores matmul so the exp lookahead never drains), the rest of
    the V projection (woven under heads 0-1), per-chunk normalization,
    and head 14's att@v inside head 15's window.
  - j8 (the 16-row key-tail tile) is packed 3 heads per [128,1024] psum
    tile at partition offsets 0/32/64 (AP base limit), one exp per
    group instead of one full-width exp per head (-12us ACT); v8's 16
    rows are replicated at those offsets so the att@v j8 stationary
    shares the moving tile's partition base (codegen requirement).
  - scores PSUM: [128,1024] 2-bank tiles + one shared tail bank for all
    16-wide query-tail strips -> 8 banks total with st 2x2 + tails 1 +
    po 3 (att@v out / norm / v-proj share the po pool; the 3rd po slot
    was worth 7us of att@v slot-chain slack).
  - normalization selector matmuls in bf16: the old fp32 ones forced the
    PE into a half-clock mode (HAM k=4) for ~48us, slowing interleaved
    bf16 matmuls from 379ns to 634ns per 512 columns.
  - otc[c] reuses qr[c]'s pool slot (qr[c]'s last reader finishes exactly
    before otc[c]'s first write), freeing 16.6KB of SBUF.
  - output projection: it-tiles 0/1 (st slots) and it2 (po slots)
    accumulate chunks 0-6 before chunk 7's reciprocal chain resolves,
    then chunk 7 joins; yps tiles created lazily so the st ring stays
    aligned; y copies alternate ACT/DVE; bf16 output over 3 queues.
  - Note: the device clock is bimodal run-to-run (~320us fast state vs
    ~386us degraded state; NEURON_RT_RESET_CORES=1 recovers a wedged
    device after an NRT fault). Timings here are fast-state.
  - DVE/gpsimd Schraudolph exp offload was tried and reverted: both
    engines' queue latency poisons the st-slot chain (+23us / +35us).
"""

import numpy as np
import ml_dtypes

B, S, D, H, HD = 8, 1040, 1024, 16, 64
PFX = 16
GRID = 32
NCHUNK = 8
NJT = 9
TAIL = S - 8 * 128  # 16
ICH3 = [(0, 512), (512, 512), (1024, 16)]
ICH2 = [(0, 512), (512, 512)]
NPT = 3  # pt ring buffers (lag-2 att@v pipeline)
BF16 = ml_dtypes.bfloat16

_compiled = None


def _rope_tables():
    half, quarter = HD // 2, HD // 4
    frac = 2.0 * np.arange(quarter, dtype=np.float64) / half
    ts = 10000.0 ** frac
    row = np.arange(GRID, dtype=np.float64)[:, None] / ts[None, :]
    row_ang = np.broadcast_to(row[:, None, :], (GRID, GRID, quarter)).reshape(
        GRID * GRID, quarter
    )
    col_ang = np.broadcast_to(row[None, :, :], (GRID, GRID, quarter)).reshape(
        GRID * GRID, quarter
    )
    cos64 = np.concatenate(
        [np.cos(row_ang).T, np.cos(row_ang).T, np.cos(col_ang).T, np.cos(col_ang).T],
        axis=0,
    )
    s64 = np.concatenate(
        [-np.sin(row_ang).T, np.sin(row_ang).T, -np.sin(col_ang).T, np.sin(col_ang).T],
        axis=0,
    )
    cosf = np.ones((HD, S), np.float64)
    sf = np.zeros((HD, S), np.float64)
    cosf[:, PFX:] = cos64
    sf[:, PFX:] = s64
    cos2 = np.concatenate([cosf, cosf], axis=0).astype(BF16)
    s2 = np.concatenate([sf, sf], axis=0).astype(BF16)
    return cos2, s2


def _swap_matrix():
    swp = np.zeros((128, 128), np.float32)
    for i in range(128):
        swp[i, i ^ 16] = 1.0
    return swp.astype(BF16)


def _build_body(nc, tc, tile, mybir, aps):
    from contextlib import ExitStack

    from concourse.alu_op_type import AluOpType

    bf = mybir.dt.bfloat16
    f32 = mybir.dt.float32
    i32 = mybir.dt.int32
    Exp = mybir.ActivationFunctionType.Exp
    # Schraudolph exp constants (folding in the 1/sqrt(HD) score scale):
    # exp(x*0.125) ~= bitcast_f32(int32(A*x + B)), rms err 1.8%
    SCHR_A = float(0.125 * (1 << 23) / np.log(2.0))
    SCHR_B = float(127.0 * (1 << 23) - 486411.0)
    SCHR_JS = ()  # DVE/gpsimd Schraudolph exp offload: tried and reverted - both engines' queue latency poisons the st->pt->att@v chain
    xT, Wq, Wk, Wv, Wo = aps["xT"], aps["Wq"], aps["Wk"], aps["Wv"], aps["Wo"]
    COS2, S2, SWP, SEL2, OUT = (
        aps["COS2"], aps["S2"], aps["SWP"], aps["SEL2"], aps["out"],
    )

    def rows_of(j):
        return 128 if j < 8 else TAIL

    with ExitStack() as ctx:
        # ---- persistent SBUF pools (live across both phases)
        p_tab = ctx.enter_context(tc.tile_pool(name="tab", bufs=1))
        p_xt = ctx.enter_context(tc.tile_pool(name="xt", bufs=1))
        p_wv = ctx.enter_context(tc.tile_pool(name="wv", bufs=1))
        p_qk = ctx.enter_context(tc.tile_pool(name="qk", bufs=24))
        p_vx = ctx.enter_context(tc.tile_pool(name="vx", bufs=9))

        sel2 = p_tab.tile([2, 128], bf, tag="sel2")
        xt = p_xt.tile([128, 8, S], bf, tag="xt")
        wv_t = p_wv.tile([128, 8, D], bf, tag="wv")

        qr = [p_qk.tile([128, S], bf, tag="qk", name=f"qr{i}") for i in range(NCHUNK)]
        # krz is 16 columns wider than S: zeroed key-columns that let the
        # group-j8 score matmuls emit 32-tall strips (16 real keys + 16
        # zero rows), so the packed psum tile needs no gap memset
        krz = [
            [p_qk.tile([128, S + 16], bf, tag="qk", name=f"krz{i}_{z}") for z in range(2)]
            for i in range(NCHUNK)
        ]
        vx = [p_vx.tile([128, 1104], bf, tag="vx", name=f"vx{i}") for i in range(NJT)]
        # otc[c] is created lazily at its first att@v write (head 2c+2's
        # weave): pool rotation then hands it qr[c]'s slot, whose last
        # reader (scores of head 2c+1) is already done - a free 16.6KB.
        otc = [None] * NCHUNK

        # ---- input DMAs, priority-ordered and queue-parallel: each DMA
        # queue moves ~105 GB/s, so the tensors needed first (xt, wq) are
        # split across two queues each; weights stream behind them on the
        # same queues in need-order (wq -> wk -> wv).
        xT3 = xT.rearrange("(k p) s -> p k s", p=128)
        nc.gpsimd.dma_start(out=xt[:, 0:3, :], in_=xT3[:, 0:3, :])
        nc.sync.dma_start(out=xt[:, 3:6, :], in_=xT3[:, 3:6, :])
        nc.scalar.dma_start(out=xt[:, 6:8, :], in_=xT3[:, 6:8, :])

        # ================= phase 1: q/k projections + RoPE =================
        with ExitStack() as p1:
            p_t1 = p1.enter_context(tc.tile_pool(name="t1", bufs=1))
            p_w = p1.enter_context(tc.tile_pool(name="w", bufs=2))
            p_tmp = p1.enter_context(tc.tile_pool(name="tmp", bufs=3))
            p_ps1 = p1.enter_context(tc.tile_pool(name="ps1", bufs=6, space="PSUM"))
            p_ps2 = p1.enter_context(tc.tile_pool(name="ps2", bufs=2, space="PSUM"))

            cos_sb = p_t1.tile([128, S], bf, tag="cos")
            s_sb = p_t1.tile([128, S], bf, tag="sin")
            swp_sb = p_t1.tile([128, 128], bf, tag="swp")
            wq_t = p_w.tile([128, 8, D], bf, tag="w", name="wq")
            wk_t = p_w.tile([128, 8, D], bf, tag="w", name="wk")
            Wq3 = Wq.rearrange("(k p) m -> p k m", p=128)
            Wk3 = Wk.rearrange("(k p) m -> p k m", p=128)
            Wv3 = Wv.rearrange("(k p) m -> p k m", p=128)
            nc.gpsimd.dma_start(out=wq_t[:, 0:3, :], in_=Wq3[:, 0:3, :])
            nc.sync.dma_start(out=wq_t[:, 3:6, :], in_=Wq3[:, 3:6, :])
            nc.scalar.dma_start(out=wq_t[:, 6:8, :], in_=Wq3[:, 6:8, :])
            nc.scalar.dma_start(out=swp_sb, in_=SWP[:, :])
            nc.scalar.dma_start(out=sel2, in_=SEL2[:, :])
            nc.gpsimd.dma_start(out=cos_sb, in_=COS2[:, :])
            nc.gpsimd.dma_start(out=s_sb, in_=S2[:, :])
            nc.gpsimd.dma_start(out=wk_t[:, 0:4, :], in_=Wk3[:, 0:4, :])
            nc.scalar.dma_start(out=wk_t[:, 4:8, :], in_=Wk3[:, 4:8, :])
            nc.gpsimd.dma_start(out=wv_t[:, 0:4, :], in_=Wv3[:, 0:4, :])
            nc.scalar.dma_start(out=wv_t[:, 4:8, :], in_=Wv3[:, 4:8, :])

            # PE warmup: scratch matmuls bring the clock up while DMAs land
            wa = p_tmp.tile([128, 512], bf, tag="wa", bufs=1)
            wb = p_tmp.tile([128, 128], bf, tag="wb", bufs=1)
            nc.vector.memset(wa, 0.0)
            nc.vector.memset(wb, 0.0)
            wps = p_ps1.tile([128, 512], f32, tag="mm1", name="warm_ps")
            for _w in range(52):
                nc.tensor.matmul(wps, wb, wa, start=True, stop=True)

            # memsets on the (otherwise idle) DVE; gpsimd only issues DMAs
            for c in range(NCHUNK):
                nc.vector.memset(krz[c][0][64:128, :], 0.0)
                nc.vector.memset(krz[c][1][0:64, :], 0.0)
                nc.gpsimd.memset(krz[c][0][0:64, 1040:1056], 0.0)
                nc.gpsimd.memset(krz[c][1][64:128, 1040:1056], 0.0)
            for j in range(NJT):
                r = rows_of(j)
                vx3 = vx[j][:, :1040].rearrange("p (h d) -> p h d", d=65)
                nc.gpsimd.memset(vx[j][:, 1040:1104], 0.0)
                nc.gpsimd.memset(vx3[:r, :, 64:65], 1.0)
                if j == 8:
                    # v8's 16 rows are replicated at partition offsets 32
                    # and 64 so the att@v j8 stationary can match the
                    # packed gpt moving tile's partition base
                    nc.gpsimd.memset(vx3[32 : 32 + r, :, 64:65], 1.0)
                    nc.gpsimd.memset(vx3[64 : 64 + r, :, 64:65], 1.0)
            # preload the exp ACT table so phase 2 doesn't pay the switch
            nc.scalar.activation(wa[0:1, 0:8], wa[0:1, 0:8], Exp, scale=0.0)

            def emit_mm1(which, w_t, c):
                raw = p_tmp.tile([128, S], bf, tag="raw", name=f"raw_{which}{c}")
                pss = [
                    p_ps1.tile([128, 512], f32, tag="mm1", name=f"mm1_{which}{c}_{i}")
                    for i in range(3)
                ]
                for k in range(8):
                    for i, (off, wdt) in enumerate(ICH3):
                        nc.tensor.matmul(
                            pss[i][:, :wdt],
                            w_t[:, k : k + 1, c * 128 : (c + 1) * 128],
                            xt[:, k : k + 1, off : off + wdt],
                            start=(k == 0),
                            stop=(k == 7),
                        )
                for i, (off, wdt) in enumerate(ICH3):
                    nc.scalar.copy(raw[:, off : off + wdt], pss[i][:, :wdt])
                return raw

            def emit_rope(which, c, raw):
                for off, wdt in ICH3:
                    sw = p_ps2.tile([128, 512], f32, tag="swp")
                    nc.tensor.matmul(
                        sw[:, :wdt], swp_sb, raw[:, off : off + wdt],
                        start=True, stop=True,
                    )
                    t2 = p_tmp.tile([128, 512], bf, tag="t2")
                    nc.vector.tensor_mul(
                        t2[:, :wdt], sw[:, :wdt], s_sb[:, off : off + wdt]
                    )
                    t1 = p_tmp.tile([128, 512], bf, tag="t1")
                    nc.vector.tensor_mul(
                        t1[:, :wdt], raw[:, off : off + wdt],
                        cos_sb[:, off : off + wdt],
                    )
                    if which == "q":
                        nc.vector.tensor_add(
                            qr[c][:, off : off + wdt], t1[:, :wdt], t2[:, :wdt]
                        )
                    else:
                        nc.vector.tensor_add(
                            krz[c][0][0:64, off : off + wdt],
                            t1[0:64, :wdt], t2[0:64, :wdt],
                        )
                        nc.vector.tensor_add(
                            krz[c][1][64:128, off : off + wdt],
                            t1[64:128, :wdt], t2[64:128, :wdt],
                        )

            steps = [("q", c) for c in range(NCHUNK)] + [
                ("k", c) for c in range(NCHUNK)
            ]
            pending = None
            for which, c in steps:
                raw = emit_mm1(which, wq_t if which == "q" else wk_t, c)
                if pending is not None:
                    emit_rope(*pending)
                pending = (which, c, raw)

            emit_rope(*pending)
            # v tiles 0-2 on phase-1 PSUM: PE work that overlaps the last
            # rope's DVE tail and the psum pool turnover barrier
            for j in range(3):
                r = rows_of(j)
                vx3 = vx[j][:, :1040].rearrange("p (h d) -> p h d", d=65)
                for ci in range(2):
                    psv = p_ps1.tile(
                        [128, 512], f32, tag="mm1", name=f"pv1_{j}_{ci}"
                    )
                    for k in range(8):
                        nc.tensor.matmul(
                            psv[:r, :],
                            xt[:, k : k + 1, j * 128 : j * 128 + r],
                            wv_t[:, k : k + 1, ci * 512 : (ci + 1) * 512],
                            start=(k == 0),
                            stop=(k == 7),
                        )
                    nc.vector.tensor_copy(
                        vx3[:r, ci * 8 : (ci + 1) * 8, 0:64],
                        psv[:r, :].rearrange("p (h d) -> p h d", h=8),
                    )

        # ============ phase 2: v-proj + attention (ACT-exp paced) ==========
        with ExitStack() as p2:
            p_wo = p2.enter_context(tc.tile_pool(name="wo", bufs=1))
            p_pt = p2.enter_context(tc.tile_pool(name="pt", bufs=NPT))
            p_cg = p2.enter_context(tc.tile_pool(name="cg", bufs=2))
            p_cs = p2.enter_context(tc.tile_pool(name="cs", bufs=2))
            p_rc = p2.enter_context(tc.tile_pool(name="rc", bufs=2))
            p_y = p2.enter_context(tc.tile_pool(name="y", bufs=2))
            p_st = p2.enter_context(tc.tile_pool(name="st", bufs=2, space="PSUM"))
            p_tl = p2.enter_context(tc.tile_pool(name="tl", bufs=1, space="PSUM"))
            p_po = p2.enter_context(tc.tile_pool(name="po", bufs=3, space="PSUM"))

            wo_t = p_wo.tile([128, 8, D], bf, tag="wo")
            Wo3 = Wo.rearrange("(k p) m -> p k m", p=128)
            nc.sync.dma_start(out=wo_t[:, 0:4, :], in_=Wo3[:, 0:4, :])
            nc.sync.dma_start(out=wo_t[:, 4:8, :], in_=Wo3[:, 4:8, :])

            pt = [
                p_pt.tile([128, 8 * S], bf, tag="pt", name=f"pt{i}")
                for i in range(NPT)
            ]
            # j8 (the 16-key tail tile) is handled per 4-head group: the 4
            # heads' [16,1040] score strips sit at partition offsets 0/32/
            # 64/96 of one shared tile so ONE exp covers all of them
            # (per-head j8 exps cost a full 1095ns for 16 rows each).
            p_gpt = p2.enter_context(tc.tile_pool(name="gpt", bufs=2))
            gpt = [
                p_gpt.tile([128, S], bf, tag="gpt", name=f"gpt{g}")
                for g in range(6)
            ]
            cs = [None] * NCHUNK  # per-chunk [2,S] denominator tiles
            p_i32 = p2.enter_context(tc.tile_pool(name="i32", bufs=1))
            # tl: [0:128) tails j0-7 (even head), [128:256) odd head,
            # [256:352) the 6 groups' j8 query-tails
            tl = p_tl.tile([128, 352], f32, tag="tl")
            nc.vector.memset(tl[:, 256:352], 0.0)

            def emit_v_tile(j):
                r = rows_of(j)
                vx3 = vx[j][:, :1040].rearrange("p (h d) -> p h d", d=65)
                for ci in range(2):
                    psv = p_po.tile([128, 512], f32, tag="po", name=f"pv{j}_{ci}")
                    for k in range(8):
                        nc.tensor.matmul(
                            psv[:r, :],
                            xt[:, k : k + 1, j * 128 : j * 128 + r],
                            wv_t[:, k : k + 1, ci * 512 : (ci + 1) * 512],
                            start=(k == 0),
                            stop=(k == 7),
                        )
                    bases = (0, 32, 64) if j == 8 else (0,)
                    for bs in bases:
                        nc.vector.tensor_copy(
                            vx3[bs : bs + r, ci * 8 : (ci + 1) * 8, 0:64],
                            psv[:r, :].rearrange("p (h d) -> p h d", h=8),
                        )

            def emit_scores_j(h, j):
                c, hb = divmod(h, 2)
                ptf = pt[h % NPT]
                tb = (h % 2) * 128
                st = p_st.tile([128, 1024], f32, tag="st", name=f"st{h}_{j}")
                for off, wdt in ICH2:
                    nc.tensor.matmul(
                        st[:, off : off + wdt],
                        krz[c][hb][:, j * 128 : (j + 1) * 128],
                        qr[c][:, off : off + wdt],
                        start=True,
                        stop=True,
                    )
                nc.tensor.matmul(
                    tl[:, tb + j * 16 : tb + (j + 1) * 16],
                    krz[c][hb][:, j * 128 : (j + 1) * 128],
                    qr[c][:, 1024:1040],
                    start=True,
                    stop=True,
                )
                if j in SCHR_JS:
                    # DVE Schraudolph exp: offloads the ACT engine, which
                    # paces the whole attention phase
                    it = p_i32.tile([128, 1024], i32, tag="i32", name=f"i{h}_{j}")
                    nc.vector.tensor_scalar(
                        it, st[:, :], SCHR_A, SCHR_B,
                        AluOpType.mult, AluOpType.add,
                    )
                    nc.gpsimd.tensor_copy(
                        ptf[:, j * S : j * S + 1024], it.bitcast(f32)
                    )
                else:
                    nc.scalar.activation(
                        ptf[:, j * S : j * S + 1024], st[:, :],
                        Exp, scale=1.0 / np.sqrt(HD),
                    )

            def emit_tails_exp(h):
                ptf = pt[h % NPT]
                ptv = ptf.rearrange("p (j q) -> p j q", q=S)
                tb = (h % 2) * 128
                nc.scalar.activation(
                    ptv[:, 0:8, 1024:1040],
                    tl[:, tb : tb + 128].rearrange("p (j t) -> p j t", t=16),
                    Exp,
                    scale=1.0 / np.sqrt(HD),
                )

            def emit_group_j8(g):
                # scores + exp for the j8 key tile of heads 3g..3g+2, packed
                # at partition offsets 32m (AP base must be 0/32/64); the
                # 32-wide stationary (16 real + 16 zero key columns) writes
                # full 32-tall strips, and rows 96:128 are never read, so
                # the packed tile needs no memset
                stg = p_st.tile([128, 1024], f32, tag="st", name=f"stg{g}")
                for m in range(3):
                    hh = 3 * g + m
                    if hh >= H:
                        break
                    c, hb = divmod(hh, 2)
                    for off, wdt in ICH2:
                        nc.tensor.matmul(
                            stg[32 * m : 32 * m + 32, off : off + wdt],
                            krz[c][hb][:, 1024:1056],
                            qr[c][:, off : off + wdt],
                            start=True,
                            stop=True,
                        )
                    nc.tensor.matmul(
                        tl[32 * m : 32 * m + 32, 256 + g * 16 : 272 + g * 16],
                        krz[c][hb][:, 1024:1056],
                        qr[c][:, 1024:1040],
                        start=True,
                        stop=True,
                    )
                nh = min(3, H - 3 * g)
                nc.scalar.activation(
                    gpt[g][0 : 32 * nh, 0:1024],
                    stg[0 : 32 * nh, :],
                    Exp,
                    scale=1.0 / np.sqrt(HD),
                )
                nc.scalar.activation(
                    gpt[g][0 : 32 * nh, 1024:1040],
                    tl[0 : 32 * nh, 256 + g * 16 : 272 + g * 16],
                    Exp,
                    scale=1.0 / np.sqrt(HD),
                )

            cstage = {}
            otd = {}

            def emit_attv_part1(h, i):
                # first half of an att@v chunk's key accumulation; split so
                # the PE block between two scores matmuls stays short enough
                # that the ACT exp stream never drains its lookahead
                c, hb = divmod(h, 2)
                off, wdt = ICH3[i]
                ptf = pt[h % NPT]
                if otc[c] is None:
                    otc[c] = p_qk.tile([128, S], bf, tag="qk", name=f"otc{c}")
                if i == 0:
                    cstage[h] = p_cg.tile([65, S], f32, tag="cg", name=f"cst{h}")
                ot = p_po.tile([128, 512], f32, tag="po", name=f"ot{h}_{i}")
                otd[(h, i)] = ot
                for j in range(4):
                    nc.tensor.matmul(
                        ot[:, :wdt],
                        vx[j][:, h * 65 : h * 65 + 128],
                        ptf[:, j * S + off : j * S + off + wdt],
                        start=(j == 0),
                        stop=False,
                    )

            def emit_attv_part2(h, i):
                c, hb = divmod(h, 2)
                off, wdt = ICH3[i]
                ptf = pt[h % NPT]
                ot = otd.pop((h, i))
                for j in range(4, 8):
                    nc.tensor.matmul(
                        ot[:, :wdt],
                        vx[j][:, h * 65 : h * 65 + 128],
                        ptf[:, j * S + off : j * S + off + wdt],
                        start=False,
                        stop=False,
                    )
                m = h % 3
                nc.tensor.matmul(
                    ot[:, :wdt],
                    vx[8][32 * m : 32 * m + TAIL, h * 65 : h * 65 + 128],
                    gpt[h // 3][32 * m : 32 * m + TAIL, off : off + wdt],
                    start=False,
                    stop=True,
                )
                nc.vector.tensor_copy(
                    otc[c][hb * 64 : hb * 64 + 64, off : off + wdt],
                    ot[0:64, :wdt],
                )
                nc.vector.tensor_copy(
                    cstage[h][64:65, off : off + wdt], ot[64:65, :wdt]
                )
                if i == 2:
                    if cs[c] is None:
                        cs[c] = p_cs.tile([2, S], f32, tag="cs", name=f"cs{c}")
                    (nc.sync if h % 2 == 0 else nc.gpsimd).dma_start(
                        out=cs[c][hb : hb + 1, :], in_=cstage[h][64:65, :]
                    )

            def emit_attv_chunk(h, i):
                emit_attv_part1(h, i)
                emit_attv_part2(h, i)

            def emit_norm(c):
                rcp = p_rc.tile([2, S], f32, tag="rcp", name=f"rcp{c}")
                nc.vector.reciprocal_approx_fast(rcp, cs[c])
                rcpb = p_rc.tile([2, S], bf, tag="rcpb", name=f"rcpb{c}")
                nc.vector.tensor_copy(rcpb, rcp)
                for off, wdt in ICH3:
                    psb = p_po.tile([128, 512], f32, tag="po", name=f"nm{c}_{off}")
                    nc.tensor.matmul(
                        psb[:, :wdt], sel2, rcpb[:, off : off + wdt],
                        start=True, stop=True,
                    )
                    nc.vector.tensor_mul(
                        otc[c][:, off : off + wdt],
                        otc[c][:, off : off + wdt],
                        psb[:, :wdt],
                    )

            # phase 1.5: rest of V projection woven under heads 0-1
            vq = [3, 4, 5, 6, 7]
            for h in (0, 1):
                for j in range(8):
                    emit_scores_j(h, j)
                    if j in (1, 3, 5) and vq:
                        emit_v_tile(vq.pop(0))
                emit_tails_exp(h)
                if h % 3 == 0:
                    emit_group_j8(h // 3)
            emit_v_tile(8)

            # steady state: head h scores woven with att@v of head h-2;
            # h15 additionally absorbs att@v(14) (its exps are done by then)
            # att@v chunks are emitted in two halves around the next scores
            # matmul so the PE block between score pairs stays short and the
            # ACT exp stream keeps its lookahead fed
            for h in range(2, H):
                for j in range(8):
                    emit_scores_j(h, j)
                    if h < 15:
                        if j in (1, 4, 6):
                            emit_attv_part1(h - 2, {1: 0, 4: 1, 6: 2}[j])
                        if j in (2, 5, 7):
                            emit_attv_part2(h - 2, {2: 0, 5: 1, 7: 2}[j])
                    else:
                        if j in (1, 4, 6):
                            emit_attv_chunk(h - 2, {1: 0, 4: 1, 6: 2}[j])
                        if j in (2, 5, 7):
                            emit_attv_chunk(14, {2: 0, 5: 1, 7: 2}[j])
                emit_tails_exp(h)
                if h % 3 == 0:
                    emit_group_j8(h // 3)
                # norm(c) needs att@v(2c+1), complete at the end of head
                # 2c+3's window -> emit at h = 2c+4; norm(6) right after
                # att@v(13) lands inside h15
                if h >= 4 and h % 2 == 0:
                    emit_norm((h - 4) // 2)
                if h == 15:
                    emit_norm(6)

            # drain: att@v for head 15, then norm(7)
            for i in range(3):
                emit_attv_chunk(15, i)

            def emit_yproj_mm(it, cs_, start, stop):
                r = rows_of(it)
                if it not in yps:
                    yps[it] = p_st.tile([128, 1024], f32, tag="st", name=f"y{it}")
                for ci in range(2):
                    for c in cs_:
                        nc.tensor.matmul(
                            yps[it][:r, ci * 512 : (ci + 1) * 512],
                            otc[c][:, it * 128 : it * 128 + r],
                            wo_t[:, c : c + 1, ci * 512 : (ci + 1) * 512],
                            start=(start and c == cs_[0]),
                            stop=(stop and c == cs_[-1]),
                        )

            def emit_yproj_out(it):
                r = rows_of(it)
                for ci in range(2):
                    ysb = p_y.tile([128, 512], bf, tag="ysb")
                    eng = (nc.scalar.copy, nc.vector.tensor_copy)[ci]
                    eng(ysb[:r, :], yps[it][:r, ci * 512 : (ci + 1) * 512])
                    qs = (
                        (nc.sync, nc.gpsimd)[ci]
                        if it >= 7
                        else (nc.sync, nc.gpsimd, nc.scalar)[(2 * it + ci) % 3]
                    )
                    qs.dma_start(
                        out=OUT[it * 128 : it * 128 + r, ci * 512 : (ci + 1) * 512],
                        in_=ysb[:r, :],
                    )

            # output projection: it0/it1 accumulate chunks 0-6 BEFORE
            # norm(7) is emitted, hiding the chunk-7 reciprocal chain
            # (cstage DMA -> rcp -> selector matmul -> DVE mul) behind PE
            # work; chunk 7 joins as the final accumulation step.
            # yps tiles are created lazily so the st-slot ring only
            # contains tiles actually used (it2 runs on po slots; an unused
            # yps[2] would misalign the ring and stall it3 behind it1)
            yps = {}
            emit_yproj_mm(0, list(range(7)), True, False)
            emit_yproj_mm(1, list(range(7)), True, False)
            # it2's partial runs on po slots (both st slots are held open by
            # it0/it1) to keep the PE fed through norm(7)'s reciprocal chain
            yp2 = [
                p_po.tile([128, 512], f32, tag="po", name=f"yp2_{ci}")
                for ci in range(2)
            ]
            for ci in range(2):
                for c in range(7):
                    nc.tensor.matmul(
                        yp2[ci][:, :],
                        otc[c][:, 2 * 128 : 3 * 128],
                        wo_t[:, c : c + 1, ci * 512 : (ci + 1) * 512],
                        start=(c == 0),
                        stop=False,
                    )
            emit_norm(7)
            emit_yproj_mm(0, [7], False, True)
            emit_yproj_out(0)
            emit_yproj_mm(1, [7], False, True)
            emit_yproj_out(1)
            for ci in range(2):
                nc.tensor.matmul(
                    yp2[ci][:, :],
                    otc[7][:, 2 * 128 : 3 * 128],
                    wo_t[:, 7:8, ci * 512 : (ci + 1) * 512],
                    start=False,
                    stop=True,
                )
                ysb = p_y.tile([128, 512], bf, tag="ysb")
                (nc.scalar.copy, nc.vector.tensor_copy)[ci](ysb, yp2[ci][:, :])
                (nc.sync, nc.gpsimd)[ci].dma_start(
                    out=OUT[2 * 128 : 3 * 128, ci * 512 : (ci + 1) * 512],
                    in_=ysb,
                )
            for it in range(3, NJT):
                emit_yproj_mm(it, list(range(8)), True, True)
                emit_yproj_out(it)


def _build():
    global _compiled
    if _compiled is not None:
        return _compiled
    import concourse.bass as bass  # noqa: F401
    import concourse.mybir as mybir
    import concourse.tile as tile
    from concourse import bacc

    nc = bacc.Bacc("TRN2", target_bir_lowering=False, debug=False)
    bf = mybir.dt.bfloat16
    aps = {
        "xT": nc.dram_tensor("xT", [D, S], bf, kind="ExternalInput").ap(),
        "Wq": nc.dram_tensor("Wq", [D, H * HD], bf, kind="ExternalInput").ap(),
        "Wk": nc.dram_tensor("Wk", [D, H * HD], bf, kind="ExternalInput").ap(),
        "Wv": nc.dram_tensor("Wv", [D, H * HD], bf, kind="ExternalInput").ap(),
        "Wo": nc.dram_tensor("Wo", [H * HD, D], bf, kind="ExternalInput").ap(),
        "COS2": nc.dram_tensor("COS2", [128, S], bf, kind="ExternalInput").ap(),
        "S2": nc.dram_tensor("S2", [128, S], bf, kind="ExternalInput").ap(),
        "SWP": nc.dram_tensor("SWP", [128, 128], bf, kind="ExternalInput").ap(),
        "SEL2": nc.dram_tensor("SEL2", [2, 128], bf, kind="ExternalInput").ap(),
        "out": nc.dram_tensor("out", [S, D], bf, kind="ExternalOutput").ap(),
    }
    with tile.TileContext(nc) as tc:
        _build_body(nc, tc, tile, mybir, aps)
    nc.compile()
    _compiled = nc
    return nc


def _install_trace_shim():
    """The agent image's antenv lacks axon_hooks, so run_bass_kernel_spmd's
    trace path can't find the NTFF profile hook trn_boot would have set.
    Recreate the module and install the ctypes hook; skip the S3 artifact
    upload (no creds needed for local timing)."""
    import sys
    import types

    if "antenv.axon_hooks" not in sys.modules:
        import antenv  # noqa: F401

        mod = types.ModuleType("antenv.axon_hooks")
        mod._hook = None

        def set_axon_ntff_profile_hook(h):
            mod._hook = h

        def get_axon_ntff_profile_hook():
            return mod._hook

        mod.set_axon_ntff_profile_hook = set_axon_ntff_profile_hook
        mod.get_axon_ntff_profile_hook = get_axon_ntff_profile_hook
        sys.modules["antenv.axon_hooks"] = mod

    import antenv.axon_hooks as ah

    if ah.get_axon_ntff_profile_hook() is None:
        from trn_agent_boot.trn_boot import _ntff_profile_via_ctypes

        ah.set_axon_ntff_profile_hook(
            _ntff_profile_via_ctypes("/opt/axon/libaxon_pjrt.so")
        )

    import concourse.bass_utils as bu

    bu.upload_artifacts = lambda tmpdir: f"local://{tmpdir}"


def run(inputs, trace=False):
    """Returns (output (8,1040,1024) f32, exec_time_ns or None)."""
    if trace:
        _install_trace_shim()
    from concourse.bass_utils import run_bass_kernel_spmd

    nc = _build()
    x = np.asarray(inputs["x"], np.float32)
    wq = np.asarray(inputs["Wq"], np.float32).reshape(D, H * HD).astype(BF16)
    wk = np.asarray(inputs["Wk"], np.float32).reshape(D, H * HD).astype(BF16)
    wv = np.asarray(inputs["Wv"], np.float32).reshape(D, H * HD).astype(BF16)
    wo = np.asarray(inputs["Wo"], np.float32).reshape(H * HD, D).astype(BF16)
    cos2, s2 = _rope_tables()
    swp = _swap_matrix()
    sel2b = np.zeros((2, 128), np.float32)
    sel2b[0, 0:64] = 1.0
    sel2b[1, 64:128] = 1.0
    shared = {
        "Wq": wq, "Wk": wk, "Wv": wv, "Wo": wo,
        "COS2": cos2, "S2": s2, "SWP": swp, "SEL2": sel2b.astype(BF16),
    }
    in_maps = [
        dict(shared, xT=np.ascontiguousarray(x[b].T).astype(BF16)) for b in range(B)
    ]
    res = run_bass_kernel_spmd(nc, in_maps, core_ids=list(range(B)), trace=trace)
    out = np.stack([np.asarray(r["out"], np.float32) for r in res.results], axis=0)
    return out, res.exec_time_ns


def kernel(x, Wq, Wk, Wv, Wo):
    out, _ = run({"x": x, "Wq": Wq, "Wk": Wk, "Wv": Wv, "Wo": Wo})
    return out
